# revision 19
# baseline (speedup 1.0000x reference)
"""Trainium2 Bass kernel for CollaborativeAttention.

Math: with S=512 unique positions and F=T=2048 gathered via fpos/tpos (mod 512),
the whole block collapses to the unique-position problem:
    qf = hs @ Wq ; kf = hs @ Wk ; vf = hs @ Wv + bv ; cbf = hs @ Wcb       [512, *]
    per head h:  w[u, s] = counts[s] * exp(scale*(qf[u]*mix[h]) . kf[s]
                                           + scale*cbf[s, h])
    ctx[u, h*64:(h+1)*64] = (w @ vf[:, h*64:(h+1)*64]) / w.sum(axis=1)
    outfull = ctx @ Wd + bd ; resfull = hs + outfull ; LN  -> normedfull   [512, 1024]
    output  = normedfull[fpos % 512]                                       [2048, 1024]
counts[s] = multiplicity of s in (tpos % 512); softmax over the 2048 keys is
exactly the count-weighted softmax over the 512 unique keys.

Distribution: collectives on this stack cost ~1.4 ms each (measured), which
dwarfs the entire collapsed computation (~0.2 ms). So every core runs the full
replicated problem (no collectives); the host takes core 0's output. Matmuls
use float32r (full-rate single-pass fp32) where the moving dim is 512.
"""

import math
import numpy as np

P = 128
S = 512
D = 1024
H = 16
DH = 64
NB = D // P          # 8 contraction chunks
N_CORES = 8
NPAIR = H // 2
SCALE = 1.0 / math.sqrt(D / H)  # 0.125
LN_EPS = 1e-5
NEG_BIG = -30000.0

# use float32r (full-rate single-pass reduced-precision fp32) on the matmul path
F32R = True

_CACHE = {}


def _emit(nc, tc, pools, io, it):
    """Emit one full compute iteration (everything after the constant loads)."""
    import concourse.mybir as mybir

    def wvblk2d(t):
        return t.rearrange("p a b -> p (a b)")[:, :S]

    fp = mybir.dt.float32
    fr = mybir.dt.float32r if F32R else fp
    Alu = mybir.AluOpType
    Act = mybir.ActivationFunctionType

    mqp, wp, wkp, ep, ps, wstream = (pools[k] for k in
                                     ("mqp", "wp", "wkp", "ep", "ps", "wstream"))
    hT = io["hT"]

    # ---- q/k projections (full):  qT = (hs @ Wq)^T, kT likewise ----
    qT = mqp.tile([P, NB, S], fr, tag="qT", name=f"qT{it}", bufs=1)
    kT = mqp.tile([P, NB, S], fr, tag="kT", name=f"kT{it}", bufs=1)
    for wdram, dest in ((io["wq"], qT), (io["wk"], kT)):
        wr = wdram.rearrange("(o p) m -> p o m", p=P)
        for o in range(NB):
            wblk = wstream.tile([P, NB, P], fr, tag="wst", name=f"wblk{it}")
            nc.sync.dma_start(wblk[:], wr[:, :, P * o: P * (o + 1)])
            pt = ps.tile([P, S], fp, tag="ps", name=f"pt{it}")
            for ic in range(NB):
                nc.tensor.matmul(pt[:], lhsT=wblk[:, ic, :],
                                 rhs=hT[:, ic, :],
                                 start=(ic == 0), stop=(ic == NB - 1))
            nc.scalar.copy(dest[:, o, :], pt[:])

    # ---- cb (content bias) for all heads: one psum bank, 4 regions ----
    cb_ps = ps.tile([P, 4, H], fp, tag="ps", name=f"cb_ps{it}")
    for ic in range(NB):
        for st in range(4):
            nc.tensor.matmul(cb_ps[:, st, :],
                             lhsT=hT[:, ic, P * st: P * (st + 1)],
                             rhs=io["wcb_sb"][:, ic, :],
                             start=(ic == 0 and st == 0),
                             stop=(ic == NB - 1 and st == 3),
                             skip_group_check=True)
    # exp bias per key s and head: scale*cb[s, h] + ln(counts[s])
    bias_sb = mqp.tile([P, 4, H], fp, tag="bias", name=f"bias_sb{it}")
    for st in range(4):
        nc.vector.scalar_tensor_tensor(
            out=bias_sb[:, st, :], in0=cb_ps[:, st, :], scalar=SCALE,
            in1=io["lncnt_sb"][:, st:st + 1].to_broadcast([P, H]),
            op0=Alu.mult, op1=Alu.add)

    # ---- v projection (full, streamed by column half) ----
    v_sb = mqp.tile([P, 4, D], fr, tag="v", name=f"v_sb{it}", bufs=1)
    wvr = io["wv"].rearrange("(o p) m -> p o m", p=P)
    for eh in range(2):
        v_ps = [ps.tile([P, S], fp, tag="ps", name=f"v_ps{it}_{st}")
                for st in range(4)]
        for ic in range(NB):
            wvblk = wstream.tile([P, NB, P], fr, tag="wst", name=f"wvblk{it}")
            nc.sync.dma_start(wvblk2d(wvblk)[:], wvr[:, ic, S * eh: S * (eh + 1)])
            for st in range(4):
                nc.tensor.matmul(v_ps[st][:],
                                 lhsT=hT[:, ic, P * st: P * (st + 1)],
                                 rhs=wvblk2d(wvblk)[:],
                                 start=(ic == 0), stop=(ic == NB - 1))
        for st in range(4):
            nc.scalar.copy(v_sb[:, st, S * eh: S * (eh + 1)], v_ps[st][:])

    # ---- per-head scores -> exp -> ctx/Z; normalize per head ----
    # mq for head h+1 is emitted BEFORE head h's ctx/z/normalize: DVE is
    # in-order, and the normalize ops wait on PE ctx completion — emitting
    # mq first keeps the next head's scores from stalling PE at each head
    # boundary.
    ctxn = mqp.tile([P, NB, S], fr, tag="ctxn", name=f"ctxn{it}", bufs=1)

    def emit_mq(h):
        mq = mqp.tile([P, NB, S], fr, tag="mq", name=f"mq{it}_{h}")
        for ic in range(NB):
            nc.vector.tensor_tensor(
                mq[:, ic, :], qT[:, ic, :],
                io["mixt_sb"][:, ic, h:h + 1].to_broadcast([P, S]),
                Alu.mult)
        return mq

    mq_next = emit_mq(0)
    for pair in range(NPAIR):
        for par in range(2):
            h = 2 * pair + par
            mq = mq_next
            w_tiles = []
            for st in range(4):
                sc = ps.tile([P, S], fp, tag="ps", name=f"sc{it}")
                for ic in range(NB):
                    nc.tensor.matmul(sc[:],
                                     lhsT=kT[:, ic, P * st: P * (st + 1)],
                                     rhs=mq[:, ic, :],
                                     start=(ic == 0), stop=(ic == NB - 1))
                wt = wp.tile([P, S], fr, tag="w", name=f"wt{it}")
                nc.scalar.activation(wt[:], sc[:], Act.Exp,
                                     bias=bias_sb[:, st, h:h + 1], scale=SCALE)
                w_tiles.append(wt)
            if h + 1 < H:
                mq_next = emit_mq(h + 1)
            # ctx: lhsT spans 128 v-columns so M=128 (fp32r needs full
            # weights); the head's real rows land at its row-half rh. z:
            # all-ones [s, 128] lhsT puts Z[u] in every output row.
            rh = DH * par
            ctx_h = ps.tile([P, S], fp, tag="ps", name=f"ctxh{it}")
            z_h = ps.tile([P, S], fp, tag="ps", name=f"zh{it}")
            for st in range(4):
                nc.tensor.matmul(ctx_h[:],
                                 lhsT=v_sb[:, st, DH * h - rh: DH * h - rh + P],
                                 rhs=w_tiles[st][:],
                                 start=(st == 0), stop=(st == 3))
                nc.tensor.matmul(z_h[:],
                                 lhsT=io["ones_sb"][:],
                                 rhs=w_tiles[st][:],
                                 start=(st == 0), stop=(st == 3))
            rz_sb = wkp.tile([P, S], fp, tag="wk", name=f"rz_sb{it}")
            nc.vector.reciprocal(rz_sb[rh:rh + DH, :], z_h[rh:rh + DH, :])
            nc.vector.tensor_tensor(ctxn[rh:rh + DH, pair, :],
                                    ctx_h[rh:rh + DH, :],
                                    rz_sb[rh:rh + DH, :], Alu.mult)
            nc.vector.tensor_scalar_add(
                ctxn[rh:rh + DH, pair, :], ctxn[rh:rh + DH, pair, :],
                io["bv_sb"][rh:rh + DH, pair:pair + 1])

    # ---- output projection (full, Wd streamed by contraction chunk) ----
    wdr = io["wd"].rearrange("(o p) m -> p o m", p=P)
    po = [ps.tile([P, S], fp, tag="ps", name=f"po{it}_{j}") for j in range(8)]
    for o in range(NB):
        wdo = wstream.tile([P, NB, P], fr, tag="wst", name=f"wdo{it}")
        wdo2 = wdo.rearrange("p a b -> p (a b)")
        nc.sync.dma_start(wdo2[:], wdr[:, o, :])
        for ut in range(4):
            for eh in range(2):
                nc.tensor.matmul(po[ut * 2 + eh][:],
                                 lhsT=ctxn[:, o, P * ut: P * (ut + 1)],
                                 rhs=wdo2[:, S * eh: S * (eh + 1)],
                                 start=(o == 0), stop=(o == NB - 1))

    # ---- epilogue: residual + bd, LayerNorm, full [512, 1024] output ----
    hidr = io["hid"].rearrange("(o p) m -> p o m", p=P)
    for ut in range(4):
        hid_t = ep.tile([P, D], fp, tag="hid", name=f"hid_t{it}")
        nc.sync.dma_start(hid_t[:], hidr[:, ut, :])
        r_sb = ep.tile([P, D], fp, tag="r", name=f"r_sb{it}")
        for eh in range(2):
            nc.vector.tensor_add(r_sb[:, S * eh: S * (eh + 1)],
                                 po[ut * 2 + eh][:],
                                 io["bd_b"][:, S * eh: S * (eh + 1)])
        nc.vector.tensor_add(r_sb[:], r_sb[:], hid_t[:])
        stats = ep.tile([P, 2, 6], fp, tag="stats", name=f"stats{it}")
        nc.vector.bn_stats(stats[:, 0, :], r_sb[:, 0:S])
        nc.vector.bn_stats(stats[:, 1, :], r_sb[:, S:D])
        mv = ep.tile([P, 2], fp, tag="mv", name=f"mv{it}")
        nc.vector.bn_aggr(mv[:], stats[:])
        std = ep.tile([P, 1], fp, tag="std", name=f"std{it}")
        nc.scalar.activation(std[:], mv[:, 1:2], Act.Sqrt,
                             bias=io["eps_t"][:], scale=1.0)
        nc.vector.reciprocal(std[:], std[:])
        nc.vector.tensor_scalar(out=r_sb[:], in0=r_sb[:],
                                scalar1=mv[:, 0:1], scalar2=std[:],
                                op0=Alu.subtract, op1=Alu.mult)
        nc.vector.tensor_tensor(r_sb[:], r_sb[:], io["gam_b"][:], Alu.mult)
        nc.vector.tensor_add(r_sb[:], r_sb[:], io["bet_b"][:])
        nc.sync.dma_start(io["out"][P * ut: P * (ut + 1), :], r_sb[:])


def _build(iters=1):
    import concourse.bass as bass
    import concourse.mybir as mybir
    import concourse.tile as tile
    from concourse import bacc

    fp = mybir.dt.float32
    fr = mybir.dt.float32r if F32R else fp

    nc = bacc.Bacc("TRN2", target_bir_lowering=False, debug=False,
                   num_devices=N_CORES)

    hiddenT = nc.dram_tensor("hiddenT", [D, S], fr, kind="ExternalInput").ap()
    hid = nc.dram_tensor("hid", [S, D], fp, kind="ExternalInput").ap()
    wq = nc.dram_tensor("wq", [D, D], fr, kind="ExternalInput").ap()
    wk = nc.dram_tensor("wk", [D, D], fr, kind="ExternalInput").ap()
    wv = nc.dram_tensor("wv", [D, D], fr, kind="ExternalInput").ap()
    wcb = nc.dram_tensor("wcb", [D, H], fr, kind="ExternalInput").ap()
    wd = nc.dram_tensor("wd", [D, D], fr, kind="ExternalInput").ap()
    mixt = nc.dram_tensor("mixt", [D, H], fr, kind="ExternalInput").ap()
    bvc = nc.dram_tensor("bvc", [D], fp, kind="ExternalInput").ap()
    lncnt = nc.dram_tensor("lncnt", [S], fp, kind="ExternalInput").ap()
    bd = nc.dram_tensor("bd", [D], fp, kind="ExternalInput").ap()
    gamma = nc.dram_tensor("gamma", [D], fp, kind="ExternalInput").ap()
    beta = nc.dram_tensor("beta", [D], fp, kind="ExternalInput").ap()
    out = nc.dram_tensor("out", [S, D], fp, kind="ExternalOutput").ap()

    def bcast_dram(vec_ap, parts):
        # DMA-replicate a [n] DRAM vector across `parts` partitions.
        return bass.AP(tensor=vec_ap.tensor, offset=vec_ap.offset,
                       ap=[[0, parts]] + [list(d) for d in vec_ap.ap])

    with tile.TileContext(nc) as tc:
        with (
            tc.tile_pool(name="singles", bufs=1) as singles,
            tc.tile_pool(name="mqp", bufs=2) as mqp,
            tc.tile_pool(name="wp", bufs=8) as wp,
            tc.tile_pool(name="wkp", bufs=4) as wkp,
            tc.tile_pool(name="ep", bufs=2) as ep,
            tc.tile_pool(name="wstream", bufs=4) as wstream,
            tc.tile_pool(name="ps", bufs=8, space="PSUM") as ps,
        ):
            pools = {"singles": singles, "mqp": mqp, "wp": wp, "wkp": wkp,
                     "ep": ep, "ps": ps, "wstream": wstream}
            # ---- constant / input loads (once) ----
            hT = singles.tile([P, NB, S], fr)
            hTr = hiddenT.rearrange("(o p) u -> p o u", p=P)
            for _o in range(NB):
                nc.sync.dma_start(hT[:, _o, :], hTr[:, _o, :])
            wcb_sb = singles.tile([P, NB, H], fr)
            nc.sync.dma_start(wcb_sb[:], wcb.rearrange("(o p) h -> p o h", p=P))
            mixt_sb = singles.tile([P, NB, H], fr)
            nc.sync.dma_start(mixt_sb[:], mixt.rearrange("(o p) h -> p o h", p=P))
            bv_sb = singles.tile([P, NB], fp)
            nc.sync.dma_start(bv_sb[:], bvc.rearrange("(o p) -> p o", p=P))
            lncnt_sb = singles.tile([P, 4], fp)
            nc.sync.dma_start(lncnt_sb[:], lncnt.rearrange("(o p) -> p o", p=P))
            bd_b = singles.tile([P, D], fp)
            nc.gpsimd.dma_start(out=bd_b[:], in_=bcast_dram(bd, P))
            gam_b = singles.tile([P, D], fp)
            nc.gpsimd.dma_start(out=gam_b[:], in_=bcast_dram(gamma, P))
            bet_b = singles.tile([P, D], fp)
            nc.gpsimd.dma_start(out=bet_b[:], in_=bcast_dram(beta, P))
            ones_f32 = singles.tile([P, P], fp)
            nc.vector.memset(ones_f32[:], 1.0)
            ones_sb = singles.tile([P, P], fr)
            with nc.allow_low_precision(reason="exact 1.0 constants to fp32r"):
                nc.vector.tensor_copy(ones_sb[:], ones_f32[:])
            eps_t = singles.tile([P, 1], fp)
            nc.vector.memset(eps_t[:], LN_EPS)

            io = {"hT": hT, "wq": wq, "wk": wk, "wv": wv, "wd": wd,
                  "wcb_sb": wcb_sb, "mixt_sb": mixt_sb, "bv_sb": bv_sb,
                  "lncnt_sb": lncnt_sb, "hid": hid, "bd_b": bd_b,
                  "gam_b": gam_b, "bet_b": bet_b, "ones_sb": ones_sb,
                  "eps_t": eps_t, "out": out}

            for it in range(iters):
                _emit(nc, tc, pools, io, it)

    nc.compile()
    return nc


def _get_nc(iters=1):
    key = ("nc", iters)
    if key not in _CACHE:
        _CACHE[key] = _build(iters)
    return _CACHE[key]


def _prepare_in_maps(hidden_states, fpos, tpos, Wq, Wk, Wcb, Wv, bv, mixing,
                     Wd, bd, ln_gamma, ln_beta):
    hs = np.ascontiguousarray(np.asarray(hidden_states, dtype=np.float32))
    tidx = np.asarray(tpos).astype(np.int64) % S
    counts = np.bincount(tidx, minlength=S).astype(np.float64)
    lncnt = np.where(counts > 0, np.log(np.maximum(counts, 1e-30)),
                     NEG_BIG).astype(np.float32)
    one = {
        "hiddenT": np.ascontiguousarray(hs.T),
        "hid": hs,
        "wq": np.ascontiguousarray(np.asarray(Wq, np.float32)),
        "wk": np.ascontiguousarray(np.asarray(Wk, np.float32)),
        "wv": np.ascontiguousarray(np.asarray(Wv, np.float32)),
        "wcb": np.ascontiguousarray(np.asarray(Wcb, np.float32)),
        "wd": np.ascontiguousarray(np.asarray(Wd, np.float32)),
        "mixt": np.ascontiguousarray(np.asarray(mixing, np.float32).T),
        "bvc": np.ascontiguousarray(np.asarray(bv, np.float32)),
        "lncnt": lncnt,
        "bd": np.ascontiguousarray(np.asarray(bd, np.float32)),
        "gamma": np.ascontiguousarray(np.asarray(ln_gamma, np.float32)),
        "beta": np.ascontiguousarray(np.asarray(ln_beta, np.float32)),
    }
    return [dict(one) for _ in range(N_CORES)]


def _run(inputs, trace=False, iters=1):
    from concourse import bass_utils
    nc = _get_nc(iters)
    in_maps = _prepare_in_maps(**inputs)
    res = bass_utils.run_bass_kernel_spmd(
        nc, in_maps, core_ids=list(range(N_CORES)), trace=trace)
    normedfull = res.results[0]["out"]
    fidx = np.asarray(inputs["fpos"]).astype(np.int64) % S
    return np.ascontiguousarray(normedfull[fidx]), res


def kernel(**inputs) -> np.ndarray:
    out, _ = _run(inputs, trace=False)
    return out

